# revision 8
# baseline (speedup 1.0000x reference)
"""Trainium2 Bass kernel for nn_LilletLayer (gnn_message_passing).

Math (per molecule b):
  xc = W_map @ x                 (coarse coords, per head h: (K=6, 3))
  delta/dist over K x K pairs -> ExpNormalSmearing -> basis (36, 50, 3)
  att[a,c,n] = sum_x basis[a,n,x]*basis[c,n,x]   (gram over pairs)
  out = silu(att @ W1 + b1) @ W2 + b2

Sharding: one NeuronCore per head h (H=8 == n_cores). Each core computes its
head's basis + gram features and contracts them with its W1 slice (folded to
the 666 upper-triangular (a,c) pairs host-side, since att is symmetric);
partial (HID, B) pre-activations are AllReduced across cores and every core
finishes silu + W2.

Device layout notes:
 - B=128 molecules sit on the 128 SBUF partitions for all elementwise stages.
 - att is produced per "a-row group" by one DVE broadcast-multiply
   (prod[c,n,x] = basis[a,n,x]*basis[c,n,x]) + one GPSIMD avg-pool over x
   (the /3 is folded into W1 host-side, as is the 2x from using
   (cos+1) instead of 0.5*(cos+1) on each gram factor).
 - att chunks are PE-transposed to [f, b], then matmul'd against streamed
   W1 tiles, accumulating h1_pre^T[j, b] in PSUM over all 278 f-chunks.
"""

import math

import numpy as np

import concourse.bacc as bacc
import concourse.bass as bass
import concourse.mybir as mybir
import concourse.tile as tile
from concourse.bass_utils import run_bass_kernel_spmd
from concourse.masks import make_identity

B, N, H, K, R = 128, 512, 8, 6, 50
CUT = 5.0
P = K * K                 # 36 (k1,k2) pairs
NPAIR = P * (P + 1) // 2  # 666 triangular (a,c) pair-pairs
FTOT = NPAIR * R          # 33300 contraction rows per head
HID = 128
F32 = mybir.dt.float32
AF = mybir.ActivationFunctionType
ALU = mybir.AluOpType

DEBUG = False


def _bcast(ap, axis, count):
    """Insert a stride-0 (broadcast) free dim at free-axis position `axis`."""
    dims = [list(d) for d in ap.ap]
    dims.insert(axis + 1, [0, count])  # +1: dims[0] is the partition dim
    return bass.AP(tensor=ap.tensor, offset=ap.offset, ap=dims)


def _with_dims(ap, dims):
    """Replace the free dims of `ap` with explicit [step, count] pairs."""
    return bass.AP(
        tensor=ap.tensor, offset=ap.offset, ap=[list(ap.ap[0])] + [list(d) for d in dims]
    )


def _mkap(ap, dims):
    """Build an AP over `ap`'s tensor with fully explicit [step, count] dims."""
    return bass.AP(tensor=ap.tensor, offset=ap.offset, ap=[list(d) for d in dims])


def build_program(n_cores=8, debug=DEBUG):
    nc = bacc.Bacc(
        "TRN2",
        target_bir_lowering=False,
        debug=False,
        enable_asserts=False,
        num_devices=n_cores,
    )

    xt = nc.dram_tensor("xt", [N, 3, B], F32, kind="ExternalInput").ap()
    wmt = nc.dram_tensor("wmt", [N, K], F32, kind="ExternalInput").ap()
    w1s = nc.dram_tensor("w1s", [FTOT, HID], F32, kind="ExternalInput").ap()
    mrep = nc.dram_tensor("mrep", [B, R], F32, kind="ExternalInput").ap()
    nbrep = nc.dram_tensor("nbrep", [B, R], F32, kind="ExternalInput").ap()
    b1d = nc.dram_tensor("b1", [HID], F32, kind="ExternalInput").ap()
    w2d = nc.dram_tensor("w2", [HID, 1], F32, kind="ExternalInput").ap()
    b2d = nc.dram_tensor("b2", [1], F32, kind="ExternalInput").ap()
    outd = nc.dram_tensor("out", [B, 1], F32, kind="ExternalOutput").ap()
    if debug:
        dbg_xc = nc.dram_tensor("dbg_xc", [B, 3, K], F32, kind="ExternalOutput").ap()
        dbg_basis = nc.dram_tensor("dbg_basis", [B, P, R, 3], F32, kind="ExternalOutput").ap()
        dbg_att0 = nc.dram_tensor("dbg_att0", [B, P * R], F32, kind="ExternalOutput").ap()
        dbg_h1 = nc.dram_tensor("dbg_h1", [HID, B], F32, kind="ExternalOutput").ap()

    with tile.TileContext(nc) as tc:
        with (
            tc.tile_pool(name="singles", bufs=1) as singles,
            tc.tile_pool(name="prodp", bufs=2) as prodp,
            tc.tile_pool(name="attp", bufs=2) as attp,
            tc.tile_pool(name="attTp", bufs=4) as attTp,
            tc.tile_pool(name="w1p", bufs=4) as w1p,
            tc.tile_pool(name="ps_t", bufs=2, space="PSUM") as ps_t_pool,
            tc.tile_pool(name="ps_acc", bufs=1, space="PSUM") as ps_acc_pool,
            tc.tile_pool(name="ps_xc", bufs=1, space="PSUM") as ps_xc_pool,
            tc.tile_pool(name="dram", bufs=1, space="DRAM") as dramp,
        ):
            # ---------------- constants / small loads ----------------
            ident = singles.tile([128, 128], F32)
            make_identity(nc, ident)

            xt_sb = singles.tile([128, 4, 3, B], F32)
            for c in range(4):
                nc.sync.dma_start(out=xt_sb[:, c], in_=xt[c * 128:(c + 1) * 128])
            wmt_sb = singles.tile([128, 4, K], F32)
            nc.sync.dma_start(
                out=wmt_sb,
                in_=_mkap(wmt, [[K, 128], [K * 128, 4], [1, K]]),
            )
            mrep_sb = singles.tile([128, R], F32)
            nc.sync.dma_start(out=mrep_sb, in_=mrep)
            nbrep_sb = singles.tile([128, R], F32)
            nc.sync.dma_start(out=nbrep_sb, in_=nbrep)
            b1_sb = singles.tile([128, 1], F32)
            nc.sync.dma_start(out=b1_sb, in_=b1d)
            w2_sb = singles.tile([128, 1], F32)
            nc.sync.dma_start(out=w2_sb, in_=w2d)
            b2_sb = singles.tile([1, 1], F32)
            nc.sync.dma_start(out=b2_sb, in_=b2d)

            # ---------------- xc = W_map @ x : [b, d, k] ----------------
            xc_sb = singles.tile([128, 3, K], F32)
            for d in range(3):
                pxc = ps_xc_pool.tile([128, K], F32, tag=f"xc{d}")
                for c in range(4):
                    nc.tensor.matmul(
                        pxc,
                        lhsT=xt_sb[:, c, d],
                        rhs=wmt_sb[:, c],
                        start=(c == 0),
                        stop=(c == 3),
                    )
                nc.scalar.copy(xc_sb[:, d], pxc)
            if debug:
                nc.sync.dma_start(out=dbg_xc, in_=xc_sb)

            # ---------------- delta[b, d, a=(k1,k2)] ----------------
            delta_sb = singles.tile([128, 3, P], F32)
            nc.vector.tensor_sub(
                delta_sb[:].rearrange("p d (i j) -> p d i j", i=K),
                _bcast(xc_sb[:], 2, K),          # [128, 3, 6, 6c] (k2 bcast)
                _bcast(xc_sb[:], 1, K),          # [128, 3, 6b, 6] (k1 bcast)
            )

            # d2[b, a] = sum_d delta^2
            d2sq_sb = singles.tile([128, P, 3], F32)
            nc.vector.tensor_mul(
                d2sq_sb,
                _with_dims(delta_sb[:], [[1, P], [P, 3]]),
                _with_dims(delta_sb[:], [[1, P], [P, 3]]),
            )
            d2_sb = singles.tile([128, P], F32)
            nc.vector.tensor_reduce(
                d2_sb, d2sq_sb, axis=mybir.AxisListType.X, op=ALU.add
            )
            dnorm_sb = singles.tile([128, P], F32)
            nc.scalar.activation(dnorm_sb, d2_sb, AF.Sqrt)

            # inv = 1/(dnorm+1e-6)^2 ; c1 = cos(min(dnorm,CUT)*pi/CUT)
            c_eps = singles.tile([128, 1], F32)
            nc.vector.memset(c_eps, 1e-6)
            c_halfpi = singles.tile([128, 1], F32)
            nc.vector.memset(c_halfpi, math.pi / 2)
            p2_sb = singles.tile([128, P], F32)
            nc.scalar.activation(p2_sb, dnorm_sb, AF.Square, bias=c_eps[:, 0:1])
            inv_sb = singles.tile([128, P], F32)
            nc.vector.reciprocal(inv_sb, p2_sb)
            dc_sb = singles.tile([128, P], F32)
            nc.vector.tensor_single_scalar(dc_sb, dnorm_sb, CUT, op=ALU.min)
            c1_sb = singles.tile([128, P], F32)
            nc.scalar.activation(
                c1_sb, dc_sb, AF.Sin, scale=-math.pi / CUT, bias=c_halfpi[:, 0:1]
            )
            # m3 = (c1 + 1) * inv   (= 2*cutoff / (d+1e-6)^2)
            m3_sb = singles.tile([128, P], F32)
            nc.vector.scalar_tensor_tensor(
                m3_sb, in0=c1_sb, scalar=1.0, in1=inv_sb, op0=ALU.add, op1=ALU.mult
            )

            # ---------------- smearing g[b, a, r] ----------------
            e_sb = singles.tile([128, P], F32)
            nc.scalar.activation(e_sb, dnorm_sb, AF.Exp, scale=-1.0)
            t_sb = singles.tile([128, P, R], F32)
            nc.vector.tensor_sub(t_sb, _bcast(e_sb[:], 1, R), _bcast(mrep_sb[:], 0, P))
            tsq_sb = singles.tile([128, P, R], F32)
            nc.vector.tensor_mul(tsq_sb, t_sb, t_sb)
            tb_sb = singles.tile([128, P, R], F32)
            nc.vector.tensor_mul(tb_sb, tsq_sb, _bcast(nbrep_sb[:], 0, P))
            g_sb = singles.tile([128, P, R], F32)
            nc.scalar.activation(g_sb, tb_sb, AF.Exp)

            # deltam[b, d, a] = delta * m3 ; basis[b, a, r, d] = deltam * g
            deltam_sb = singles.tile([128, 3, P], F32)
            nc.vector.tensor_mul(deltam_sb, delta_sb, _bcast(m3_sb[:], 0, 3))
            basis_sb = singles.tile([128, P, R, 3], F32)
            nc.vector.tensor_mul(
                basis_sb,
                _with_dims(deltam_sb[:], [[1, P], [0, R], [P, 3]]),
                _with_dims(g_sb[:], [[R, P], [1, R], [0, 3]]),
            )
            if debug:
                nc.sync.dma_start(out=dbg_basis, in_=basis_sb)

            # ---------------- att + big contraction ----------------
            ps_acc = ps_acc_pool.tile([HID, B], F32)
            n_mms = sum(
                (v + 127) // 128 for v in ((P - a) * R for a in range(P))
            )
            mm = 0
            fbase = 0
            for a in range(P):
                cc = P - a
                span = cc * R
                prod_t = prodp.tile([128, 3, cc, R], F32, tag="prod")
                nc.vector.tensor_mul(
                    prod_t,
                    _with_dims(
                        basis_sb[:, a], [[1, 3], [0, cc], [3, R]]
                    ),
                    _with_dims(
                        basis_sb[:, a], [[1, 3], [3 * R, cc], [3, R]]
                    ),
                )
                att_t = attp.tile([128, cc, R], F32, tag="att")
                nc.gpsimd.tensor_add(att_t, prod_t[:, 0], prod_t[:, 1])
                nc.gpsimd.tensor_add(att_t, att_t, prod_t[:, 2])
                if debug and a == 0:
                    nc.sync.dma_start(out=dbg_att0, in_=att_t)
                att_flat = att_t[:].rearrange("p c r -> p (c r)")
                for off in range(0, span, 128):
                    kk = min(128, span - off)
                    pst = ps_t_pool.tile([128, B], F32, tag="pst")
                    nc.tensor.transpose(
                        pst[:kk], att_flat[:, off:off + kk], ident
                    )
                    attT_t = attTp.tile([128, B], F32, tag="attT")
                    nc.scalar.copy(attT_t[:kk], pst[:kk])
                    w1_t = w1p.tile([128, HID], F32, tag="w1")
                    nc.sync.dma_start(
                        out=w1_t[:kk], in_=w1s[fbase + off:fbase + off + kk]
                    )
                    nc.tensor.matmul(
                        ps_acc,
                        lhsT=w1_t[:kk],
                        rhs=attT_t[:kk],
                        start=(mm == 0),
                        stop=(mm == n_mms - 1),
                    )
                    mm += 1
                fbase += span
            assert mm == n_mms and fbase == FTOT

            # ---------------- all-reduce + head ----------------
            h1p_sb = singles.tile([HID, B], F32)
            nc.scalar.copy(h1p_sb, ps_acc)
            if debug:
                nc.sync.dma_start(out=dbg_h1, in_=h1p_sb)
            ar_in = dramp.tile([HID, B], F32, tag="ar_in")
            ar_out = dramp.tile([HID, B], F32, tag="ar_out")
            nc.sync.dma_start(out=ar_in, in_=h1p_sb)
            nc.gpsimd.collective_compute(
                "AllReduce",
                ALU.add,
                replica_groups=[list(range(n_cores))],
                ins=[ar_in[:].opt()],
                outs=[ar_out[:].opt()],
            )
            h1r_sb = singles.tile([HID, B], F32)
            nc.sync.dma_start(out=h1r_sb, in_=ar_out)
            hb_sb = singles.tile([HID, B], F32)
            nc.scalar.activation(hb_sb, h1r_sb, AF.Identity, bias=b1_sb[:, 0:1])
            sg_sb = singles.tile([HID, B], F32)
            nc.scalar.activation(sg_sb, hb_sb, AF.Sigmoid)
            s_sb = singles.tile([HID, B], F32)
            nc.vector.tensor_mul(s_sb, hb_sb, sg_sb)
            ps_o = ps_xc_pool.tile([1, B], F32, tag="po")
            nc.tensor.matmul(ps_o, lhsT=w2_sb, rhs=s_sb, start=True, stop=True)
            out_sb = singles.tile([1, B], F32)
            nc.scalar.activation(
                out_sb, ps_o, AF.Identity, bias=b2_sb[0:1, 0:1]
            )
            nc.sync.dma_start(out=outd, in_=out_sb)

    nc.compile()
    return nc


def host_prep(x, W_map, means, betas, W1, b1, W2, b2):
    """Build the 8 per-core input maps (numpy, all float32)."""
    x = np.ascontiguousarray(np.asarray(x, np.float32))
    W_map = np.asarray(W_map, np.float32)
    means = np.asarray(means, np.float32)
    betas = np.asarray(betas, np.float32)
    W1 = np.asarray(W1, np.float32)
    b1 = np.ascontiguousarray(np.asarray(b1, np.float32))
    W2 = np.ascontiguousarray(np.asarray(W2, np.float32).reshape(HID, 1))
    b2 = np.ascontiguousarray(np.asarray(b2, np.float32).reshape(1))

    xT = np.ascontiguousarray(x.transpose(1, 2, 0))  # (N, 3, B)
    tri_a, tri_c = np.triu_indices(P)
    W1r = W1.reshape(H, P, P, R, HID)
    W1s = W1r[:, tri_a, tri_c] + np.where(
        (tri_a != tri_c)[None, :, None, None], W1r[:, tri_c, tri_a], 0.0
    )
    # x0.25: device gram factors are 2x ref (cutoff computed as cos+1)
    W1s_dev = np.ascontiguousarray((W1s * 0.25).reshape(H, FTOT, HID), np.float32)
    mrep = np.ascontiguousarray(np.broadcast_to(means, (B, R)), np.float32)
    nbrep = np.ascontiguousarray(np.broadcast_to(-betas, (B, R)), np.float32)

    in_maps = []
    for h in range(H):
        in_maps.append(
            dict(
                xt=xT,
                wmt=np.ascontiguousarray(W_map[h].T),  # (N, K)
                w1s=W1s_dev[h],
                mrep=mrep,
                nbrep=nbrep,
                b1=b1,
                w2=W2,
                b2=b2,
            )
        )
    return in_maps


_NC_CACHE = {}


def get_program(debug=DEBUG):
    key = bool(debug)
    if key not in _NC_CACHE:
        _NC_CACHE[key] = build_program(debug=debug)
    return _NC_CACHE[key]


def kernel(x, W_map, means, betas, W1, b1, W2, b2, _debug=False, _trace=False):
    in_maps = host_prep(x, W_map, means, betas, W1, b1, W2, b2)
    nc = get_program(debug=_debug)
    res = run_bass_kernel_spmd(nc, in_maps, list(range(H)), trace=_trace)
    out = np.asarray(res.results[0]["out"], np.float32)
    if _debug or _trace:
        kernel.last_results = res
    return out


# revision 12
# speedup vs baseline: 1.8315x; 1.8315x over previous
"""Trainium2 Bass kernel for nn_LilletLayer (gnn_message_passing).

Math (per molecule b, per head h):
  xc = W_map @ x   (6 coarse particles, 3d coords)
  delta over 36 (k1,k2) pairs -> ExpNormalSmearing -> basis (36, 50, 3)
  att[a,c,n] = sum_x basis[a,n,x]*basis[c,n,x]
  out = silu(att @ W1 + b1) @ W2 + b2

Key factorization: basis[a,n,x] = deltam[x,a] * g[a,n] is separable, so
  att[a,c,n] = D2[a,c] * g[a,n] * g[c,n],  D2 = deltam^T deltam (36x36)
— no (a,c,n,x) product tensor is ever needed; att is two broadcast
multiplies per a-row over the upper-triangular (a,c) pairs (att is
symmetric: W1 is folded to 666 pairs host-side).

Sharding: one NeuronCore per head (H=8). Each core: basis/gram for its
head, bf16 att tiles PE-transposed to [f, b], matmul'd against streamed
bf16 W1 tiles accumulating h1_pre^T[j,b] fp32 in PSUM; AllReduce across
cores; every core finishes silu + W2.

Layout: B=128 molecules on the 128 SBUF partitions for all elementwise
stages; f-rows on partitions for the contraction (via PE transposes).
"""

import math

import numpy as np

import concourse.bacc as bacc
import concourse.bass as bass
import concourse.mybir as mybir
import concourse.tile as tile
from concourse.bass_utils import run_bass_kernel_spmd
from concourse.masks import make_identity

B, N, H, K, R = 128, 512, 8, 6, 50
CUT = 5.0
P = K * K                 # 36 (k1,k2) pairs
NPAIR = P * (P + 1) // 2  # 666 triangular (a,c) pair-pairs
FTOT = NPAIR * R          # 33300 contraction rows per head
HID = 128
F32 = mybir.dt.float32
BF16 = mybir.dt.bfloat16
AF = mybir.ActivationFunctionType
ALU = mybir.AluOpType

DEBUG = False


def _bcast(ap, axis, count):
    """Insert a stride-0 (broadcast) free dim at free-axis position `axis`."""
    dims = [list(d) for d in ap.ap]
    dims.insert(axis + 1, [0, count])  # +1: dims[0] is the partition dim
    return bass.AP(tensor=ap.tensor, offset=ap.offset, ap=dims)


def _with_dims(ap, dims):
    """Replace the free dims of `ap` with explicit [step, count] pairs."""
    return bass.AP(
        tensor=ap.tensor, offset=ap.offset, ap=[list(ap.ap[0])] + [list(d) for d in dims]
    )


def _mkap(ap, dims):
    """Build an AP over `ap`'s tensor with fully explicit [step, count] dims."""
    return bass.AP(tensor=ap.tensor, offset=ap.offset, ap=[list(d) for d in dims])


def build_program(n_cores=8, debug=DEBUG):
    nc = bacc.Bacc(
        "TRN2",
        target_bir_lowering=False,
        debug=False,
        enable_asserts=False,
        num_devices=n_cores,
    )

    xt = nc.dram_tensor("xt", [N, 3, B], F32, kind="ExternalInput").ap()
    wmt = nc.dram_tensor("wmt", [N, K], F32, kind="ExternalInput").ap()
    w1s = nc.dram_tensor("w1s", [FTOT, HID], BF16, kind="ExternalInput").ap()
    mrep = nc.dram_tensor("mrep", [B, R], F32, kind="ExternalInput").ap()
    nbrep = nc.dram_tensor("nbrep", [B, R], F32, kind="ExternalInput").ap()
    b1d = nc.dram_tensor("b1", [HID], F32, kind="ExternalInput").ap()
    w2d = nc.dram_tensor("w2", [HID, 1], F32, kind="ExternalInput").ap()
    b2d = nc.dram_tensor("b2", [1], F32, kind="ExternalInput").ap()
    outd = nc.dram_tensor("out", [B, 1], F32, kind="ExternalOutput").ap()
    if debug:
        dbg_xc = nc.dram_tensor("dbg_xc", [B, 3, K], F32, kind="ExternalOutput").ap()
        dbg_g = nc.dram_tensor("dbg_g", [B, P, R], F32, kind="ExternalOutput").ap()
        dbg_d2f = nc.dram_tensor("dbg_d2f", [B, P, P], F32, kind="ExternalOutput").ap()
        dbg_att0 = nc.dram_tensor("dbg_att0", [B, P * R], F32, kind="ExternalOutput").ap()
        dbg_h1 = nc.dram_tensor("dbg_h1", [HID, B], F32, kind="ExternalOutput").ap()

    with tile.TileContext(nc) as tc:
        with (
            tc.tile_pool(name="singles", bufs=1) as singles,
            tc.tile_pool(name="g2p", bufs=2) as g2p,
            tc.tile_pool(name="attp", bufs=2) as attp,
            tc.tile_pool(name="attTp", bufs=3) as attTp,
            tc.tile_pool(name="w1p", bufs=2) as w1p,
            tc.tile_pool(name="ps_t", bufs=2, space="PSUM") as ps_t_pool,
            tc.tile_pool(name="ps_acc", bufs=1, space="PSUM") as ps_acc_pool,
            tc.tile_pool(name="ps_xc", bufs=1, space="PSUM") as ps_xc_pool,
            tc.tile_pool(name="dram", bufs=1, space="DRAM") as dramp,
        ):
            # ---------------- constants / small loads ----------------
            ident = singles.tile([128, 128], BF16)
            make_identity(nc, ident)

            xt_sb = singles.tile([128, 4, 3, B], F32)
            for c in range(4):
                nc.sync.dma_start(out=xt_sb[:, c], in_=xt[c * 128:(c + 1) * 128])
            wmt_sb = singles.tile([128, 4, K], F32)
            nc.sync.dma_start(
                out=wmt_sb,
                in_=_mkap(wmt, [[K, 128], [K * 128, 4], [1, K]]),
            )
            mrep_sb = singles.tile([128, R], F32)
            nc.sync.dma_start(out=mrep_sb, in_=mrep)
            nbrep_sb = singles.tile([128, R], F32)
            nc.sync.dma_start(out=nbrep_sb, in_=nbrep)
            b1_sb = singles.tile([128, 1], F32)
            nc.sync.dma_start(out=b1_sb, in_=b1d)
            w2_sb = singles.tile([128, 1], F32)
            nc.sync.dma_start(out=w2_sb, in_=w2d)
            b2_sb = singles.tile([1, 1], F32)
            nc.sync.dma_start(out=b2_sb, in_=b2d)

            # ---------------- xc = W_map @ x : [b, d, k] ----------------
            xc_sb = singles.tile([128, 3, K], F32)
            for d in range(3):
                pxc = ps_xc_pool.tile([128, K], F32, tag=f"xc{d}")
                for c in range(4):
                    nc.tensor.matmul(
                        pxc,
                        lhsT=xt_sb[:, c, d],
                        rhs=wmt_sb[:, c],
                        start=(c == 0),
                        stop=(c == 3),
                    )
                nc.scalar.copy(xc_sb[:, d], pxc)
            if debug:
                nc.sync.dma_start(out=dbg_xc, in_=xc_sb)

            # ---------------- delta[b, d, a=(k1,k2)] ----------------
            delta_sb = singles.tile([128, 3, P], F32)
            nc.vector.tensor_sub(
                delta_sb[:].rearrange("p d (i j) -> p d i j", i=K),
                _bcast(xc_sb[:], 2, K),          # [128, 3, 6, 6c] (k2 bcast)
                _bcast(xc_sb[:], 1, K),          # [128, 3, 6b, 6] (k1 bcast)
            )

            # d2[b, a] = sum_d delta^2
            d2sq_sb = singles.tile([128, P, 3], F32)
            nc.vector.tensor_mul(
                d2sq_sb,
                _with_dims(delta_sb[:], [[1, P], [P, 3]]),
                _with_dims(delta_sb[:], [[1, P], [P, 3]]),
            )
            d2_sb = singles.tile([128, P], F32)
            nc.vector.tensor_reduce(
                d2_sb, d2sq_sb, axis=mybir.AxisListType.X, op=ALU.add
            )
            dnorm_sb = singles.tile([128, P], F32)
            nc.scalar.activation(dnorm_sb, d2_sb, AF.Sqrt)

            # inv = 1/(dnorm+1e-6)^2 ; c1 = cos(min(dnorm,CUT)*pi/CUT)
            c_eps = singles.tile([128, 1], F32)
            nc.vector.memset(c_eps, 1e-6)
            c_halfpi = singles.tile([128, 1], F32)
            nc.vector.memset(c_halfpi, math.pi / 2)
            p2_sb = singles.tile([128, P], F32)
            nc.scalar.activation(p2_sb, dnorm_sb, AF.Square, bias=c_eps[:, 0:1])
            inv_sb = singles.tile([128, P], F32)
            nc.vector.reciprocal(inv_sb, p2_sb)
            dc_sb = singles.tile([128, P], F32)
            nc.vector.tensor_single_scalar(dc_sb, dnorm_sb, CUT, op=ALU.min)
            c1_sb = singles.tile([128, P], F32)
            nc.scalar.activation(
                c1_sb, dc_sb, AF.Sin, scale=-math.pi / CUT, bias=c_halfpi[:, 0:1]
            )
            # m3 = (c1 + 1) * inv   (= 2*cutoff / (d+1e-6)^2)
            m3_sb = singles.tile([128, P], F32)
            nc.vector.scalar_tensor_tensor(
                m3_sb, in0=c1_sb, scalar=1.0, in1=inv_sb, op0=ALU.add, op1=ALU.mult
            )

            # ---------------- smearing g[b, a, r] (bf16) ----------------
            e_sb = singles.tile([128, P], F32)
            nc.scalar.activation(e_sb, dnorm_sb, AF.Exp, scale=-1.0)
            t_sb = singles.tile([128, P, R], F32)
            nc.vector.tensor_sub(t_sb, _bcast(e_sb[:], 1, R), _bcast(mrep_sb[:], 0, P))
            tsq_sb = singles.tile([128, P, R], F32)
            nc.vector.tensor_mul(tsq_sb, t_sb, t_sb)
            tb_sb = singles.tile([128, P, R], F32)
            nc.vector.tensor_mul(tb_sb, tsq_sb, _bcast(nbrep_sb[:], 0, P))
            g_sb = singles.tile([128, P, R], BF16)
            nc.scalar.activation(g_sb, tb_sb, AF.Exp)
            if debug:
                gdbg = singles.tile([128, P, R], F32)
                nc.vector.tensor_copy(gdbg, g_sb)
                nc.sync.dma_start(out=dbg_g, in_=gdbg)

            # deltam[b, d, a] = delta * m3 ; D2f[b, A, C] = sum_x dm[x,A]dm[x,C]
            deltam_sb = singles.tile([128, 3, P], F32)
            nc.vector.tensor_mul(deltam_sb, delta_sb, _bcast(m3_sb[:], 0, 3))
            q0 = singles.tile([128, P, P], F32)
            q1 = singles.tile([128, P, P], F32)
            nc.vector.tensor_mul(
                q0,
                _with_dims(deltam_sb[:, 0], [[1, P], [0, P]]),
                _with_dims(deltam_sb[:, 0], [[0, P], [1, P]]),
            )
            nc.vector.tensor_mul(
                q1,
                _with_dims(deltam_sb[:, 1], [[1, P], [0, P]]),
                _with_dims(deltam_sb[:, 1], [[0, P], [1, P]]),
            )
            q01 = singles.tile([128, P, P], F32)
            nc.vector.tensor_add(q01, q0, q1)
            q2 = singles.tile([128, P, P], F32)
            nc.vector.tensor_mul(
                q2,
                _with_dims(deltam_sb[:, 2], [[1, P], [0, P]]),
                _with_dims(deltam_sb[:, 2], [[0, P], [1, P]]),
            )
            d2f_sb = singles.tile([128, P, P], BF16)
            nc.vector.tensor_add(d2f_sb, q01, q2)
            if debug:
                ddbg = singles.tile([128, P, P], F32)
                nc.vector.tensor_copy(ddbg, d2f_sb)
                nc.sync.dma_start(out=dbg_d2f, in_=ddbg)

            # ---------------- att + big contraction ----------------
            ps_acc = ps_acc_pool.tile([HID, B], F32)
            n_mms = sum(
                ((P - a) * R + 127) // 128 for a in range(P)
            )
            mm = 0
            fbase = 0
            for a in range(P):
                cc = P - a
                span = cc * R
                nch = (span + 127) // 128
                # att rows for this a: att[c', n] = g[a,n]*g[c,n] * D2[a,c]
                g2_t = g2p.tile([128, cc, R], BF16, tag="g2")
                nc.vector.tensor_mul(
                    g2_t,
                    _with_dims(g_sb[:, a], [[0, cc], [1, R]]),
                    _with_dims(g_sb[:, a], [[R, cc], [1, R]]),
                )
                att_t = attp.tile([128, cc, R], BF16, tag="att")
                nc.vector.tensor_mul(
                    att_t,
                    g2_t,
                    _with_dims(d2f_sb[:, a, a:], [[1, cc], [0, R]]),
                )
                if debug and a == 0:
                    adbg = singles.tile([128, P * R], F32)
                    nc.vector.tensor_copy(
                        adbg, att_t[:].rearrange("p c r -> p (c r)")
                    )
                    nc.sync.dma_start(out=dbg_att0, in_=adbg)
                att_flat = att_t[:].rearrange("p c r -> p (c r)")

                # W1 rows for the whole a-group in (at most) two DMAs
                w1g = w1p.tile([128, nch, HID], BF16, tag="w1")
                ntf = span // 128
                rem = span - ntf * 128
                if ntf:
                    nc.sync.dma_start(
                        out=w1g[:, :ntf],
                        in_=_mkap(
                            w1s[fbase:fbase + span],
                            [[HID, 128], [HID * 128, ntf], [1, HID]],
                        ),
                    )
                if rem:
                    nc.sync.dma_start(
                        out=w1g[:rem, ntf],
                        in_=w1s[fbase + ntf * 128:fbase + span],
                    )

                # bundles of up to 4 chunks -> one PSUM bank + one copy
                for c0 in range(0, nch, 4):
                    nb = min(4, nch - c0)
                    kks = [
                        min(128, span - (c0 + i) * 128) for i in range(nb)
                    ]
                    pst = ps_t_pool.tile([128, 4, B], BF16, tag="pst")
                    for i in range(nb):
                        off = (c0 + i) * 128
                        nc.tensor.transpose(
                            pst[:kks[i], i], att_flat[:, off:off + kks[i]], ident
                        )
                    attT_t = attTp.tile([128, 4, B], BF16, tag="attT")
                    nfull = sum(1 for v in kks if v == 128)
                    if nfull:
                        nc.scalar.copy(attT_t[:, :nfull], pst[:, :nfull])
                    if nfull < nb:
                        nc.scalar.copy(
                            attT_t[:kks[nfull], nfull], pst[:kks[nfull], nfull]
                        )
                    for i in range(nb):
                        off = (c0 + i) * 128
                        kk = min(128, span - off)
                        nc.tensor.matmul(
                            ps_acc,
                            lhsT=w1g[:kk, c0 + i],
                            rhs=attT_t[:kk, i],
                            start=(mm == 0),
                            stop=(mm == n_mms - 1),
                        )
                        mm += 1
                fbase += span
            assert mm == n_mms and fbase == FTOT

            # ---------------- all-reduce + head ----------------
            h1p_sb = singles.tile([HID, B], F32)
            nc.scalar.copy(h1p_sb, ps_acc)
            if debug:
                nc.sync.dma_start(out=dbg_h1, in_=h1p_sb)
            ar_in = dramp.tile([HID, B], F32, tag="ar_in")
            ar_out = dramp.tile([HID, B], F32, tag="ar_out")
            nc.sync.dma_start(out=ar_in, in_=h1p_sb)
            nc.gpsimd.collective_compute(
                "AllReduce",
                ALU.add,
                replica_groups=[list(range(n_cores))],
                ins=[ar_in[:].opt()],
                outs=[ar_out[:].opt()],
            )
            h1r_sb = singles.tile([HID, B], F32)
            nc.sync.dma_start(out=h1r_sb, in_=ar_out)
            hb_sb = singles.tile([HID, B], F32)
            nc.scalar.activation(hb_sb, h1r_sb, AF.Identity, bias=b1_sb[:, 0:1])
            sg_sb = singles.tile([HID, B], F32)
            nc.scalar.activation(sg_sb, hb_sb, AF.Sigmoid)
            s_sb = singles.tile([HID, B], F32)
            nc.vector.tensor_mul(s_sb, hb_sb, sg_sb)
            ps_o = ps_xc_pool.tile([1, B], F32, tag="po")
            nc.tensor.matmul(ps_o, lhsT=w2_sb, rhs=s_sb, start=True, stop=True)
            out_sb = singles.tile([1, B], F32)
            nc.scalar.activation(
                out_sb, ps_o, AF.Identity, bias=b2_sb[0:1, 0:1]
            )
            nc.sync.dma_start(out=outd, in_=out_sb)

    nc.compile()
    return nc


def host_prep(x, W_map, means, betas, W1, b1, W2, b2):
    """Build the 8 per-core input maps (numpy)."""
    import ml_dtypes

    x = np.ascontiguousarray(np.asarray(x, np.float32))
    W_map = np.asarray(W_map, np.float32)
    means = np.asarray(means, np.float32)
    betas = np.asarray(betas, np.float32)
    W1 = np.asarray(W1, np.float32)
    b1 = np.ascontiguousarray(np.asarray(b1, np.float32))
    W2 = np.ascontiguousarray(np.asarray(W2, np.float32).reshape(HID, 1))
    b2 = np.ascontiguousarray(np.asarray(b2, np.float32).reshape(1))

    xT = np.ascontiguousarray(x.transpose(1, 2, 0))  # (N, 3, B)
    tri_a, tri_c = np.triu_indices(P)
    W1r = W1.reshape(H, P, P, R, HID)
    W1s = W1r[:, tri_a, tri_c] + np.where(
        (tri_a != tri_c)[None, :, None, None], W1r[:, tri_c, tri_a], 0.0
    )
    # x0.25: device gram factors are 2x ref (cutoff computed as cos+1)
    W1s_dev = np.ascontiguousarray(
        (W1s * 0.25).reshape(H, FTOT, HID).astype(ml_dtypes.bfloat16)
    )
    mrep = np.ascontiguousarray(np.broadcast_to(means, (B, R)), np.float32)
    nbrep = np.ascontiguousarray(np.broadcast_to(-betas, (B, R)), np.float32)

    in_maps = []
    for h in range(H):
        in_maps.append(
            dict(
                xt=xT,
                wmt=np.ascontiguousarray(W_map[h].T),  # (N, K)
                w1s=W1s_dev[h],
                mrep=mrep,
                nbrep=nbrep,
                b1=b1,
                w2=W2,
                b2=b2,
            )
        )
    return in_maps


_NC_CACHE = {}


def get_program(debug=DEBUG):
    key = bool(debug)
    if key not in _NC_CACHE:
        _NC_CACHE[key] = build_program(debug=debug)
    return _NC_CACHE[key]


def kernel(x, W_map, means, betas, W1, b1, W2, b2, _debug=False, _trace=False):
    in_maps = host_prep(x, W_map, means, betas, W1, b1, W2, b2)
    nc = get_program(debug=_debug)
    res = run_bass_kernel_spmd(nc, in_maps, list(range(H)), trace=_trace)
    out = np.asarray(res.results[0]["out"], np.float32)
    if _debug or _trace:
        kernel.last_results = res
    return out
